# revision 29
# baseline (speedup 1.0000x reference)
"""Trainium2 Bass kernel for nn_MultiHeadAttention_68659347194437.

Spatial attention over the W axis (no softmax) with 1x1-conv projections:
    qp = wq*q + bq ; kp, vp likewise            (C=32 channels)
    attn = qp @ kp^T  per (b,h)                 [512, 512]
    att  = attn @ vp                            [512, 32]
    out  = att^T + q                            (NCHW residual)

No softmax -> the [512,512] score matrix collapses associatively:
    out_h = (A @ M1_h + [I;0])^T @ Qaug_h,   per head h, where
    A  = [wq|bq]^T [wk|bk]                 [33,33]  (host-precomputed)
    M1 = Gt^T Pv^T                         [33,32]  (Gt = Vaug Kaug^T)
The +I folds the residual; row 32 of A@M1 is a per-head output bias.

Device schedule: 8 pipeline iterations of 2 blocks (8 heads). Per block
the final matmul is ONE K=128 matmul with a block-diagonal [128,128]
weight (4 heads stacked) streaming a [128,512] stacked-q tile. The
block-diag weight PSUM is initialized with an identity matmul (zeros
off-diagonal + the residual +I in one PE op), so each PSUM->SBUF drain
is a single wide copy per pair -- DVE ops have ~300ns fixed cost, so
copies are batched aggressively. The per-head bias column rides in the
same copy and feeds the scalar-engine ACTIVATE that drains the output.

Sharding: data-parallel over batch B=8 across 8 NeuronCores, no comms.
Host-side work is pure relayout/packing of inputs and tiny weight algebra.
"""

import os
import numpy as np

import concourse.bass as bass
import concourse.bacc as bacc
import concourse.tile as tile
import concourse.mybir as mybir
from concourse.bass_utils import run_bass_kernel_spmd

B, C, H, W = 8, 32, 64, 512
CA = C + 1           # 33, augmented channel dim
NBLK = H // 4        # 16 blocks of 4 heads
NPAIR = NBLK // 2    # 8 pipeline iterations of 2 blocks
NCH = 4 * 4          # chunks per block (4 heads x 4 chunks of 128 pixels)
PAIR_GROUPS = [1, 1, 2, 2, 2]        # DMA group sizes in block-pairs
PIPE = 3             # out-stage lags G-stage by 3 iterations

last_exec_time_ns = None

_cache = {}

_BF16_NP = np.dtype(mybir.dt.np(mybir.dt.bfloat16))


def _build():
    bf16 = mybir.dt.bfloat16
    f32 = mybir.dt.float32

    nc = bacc.Bacc(
        "TRN2",
        target_bir_lowering=False,
        debug=False,
        enable_asserts=False,
        num_devices=8,
    )

    qs_d = nc.dram_tensor("qs", [128, NBLK * W], bf16, kind="ExternalInput")
    # k/v merged per DMA-group: [kt_group | vt_group] -- one DMA stream
    # feeds both G operands while keeping them in separate SBUF regions
    kv_d = nc.dram_tensor("kv", [128, NBLK * NCH * 2 * CA], bf16, kind="ExternalInput")
    # packed consts: cols 0:32 awt (A[0:32,:]^T), 32 alast (A[32,:]),
    # 33:65 pvt ([wv^T; bv]), 65:329 [I128|0|I128|0] (132-col stride)
    cst_d = nc.dram_tensor("cst", [128, 329], bf16, kind="ExternalInput")
    out_d = nc.dram_tensor("out", [128, NBLK * W], bf16, kind="ExternalOutput")

    qs_a = qs_d.ap()
    kv_a = kv_d.ap()
    out_a = out_d.ap()

    gp0 = []
    acc = 0
    for n in PAIR_GROUPS:
        gp0.append(acc)
        acc += n
    pair2g = []
    for g, n in enumerate(PAIR_GROUPS):
        pair2g += [g] * n

    with tile.TileContext(nc) as tc:
        with (
            tc.tile_pool(name="const", bufs=1) as cpool,
            tc.tile_pool(name="inp", bufs=1) as inpool,
            tc.tile_pool(name="small", bufs=3) as spool,
            tc.tile_pool(name="outp", bufs=4) as opool,
            tc.tile_pool(name="psg", bufs=2, space=bass.MemorySpace.PSUM) as psg,
            tc.tile_pool(name="psm", bufs=1, space=bass.MemorySpace.PSUM) as psm,
            tc.tile_pool(name="psw", bufs=2, space=bass.MemorySpace.PSUM) as psw,
            tc.tile_pool(name="pso", bufs=3, space=bass.MemorySpace.PSUM) as pso,
        ):
            cst = cpool.tile([128, 329], bf16)
            awt = cst[0:33, 0:32]       # [33, 32] (A_w^T)
            alast = cst[0:33, 32:33]    # [33, 1]
            pvt = cst[0:33, 33:65]      # [33, 32]
            i128 = cst[:, 65:193]       # [128, 128]
            ii2 = cst[:, 65:329]        # [128, 264]: [I128|0|I128|0]

            # ---- input DMA kicks, all up front, in need order ----
            # sync: kv0, kv2, kv4, qsB;  scalar: cst, kv1, qsA, kv3
            kvg = []
            for g, n in enumerate(PAIR_GROUPS):
                cn = n * 2 * NCH * 2 * CA
                t = inpool.tile([128, cn], bf16, tag=f"kvg{g}", name=f"kvg{g}")
                kvg.append(t)
            # qs in 3 slabs: blocks 0-3, 4-11, 12-15
            QS_SLABS = [(0, 4), (4, 8), (12, 4)]
            qsg = [
                inpool.tile([128, n * W], bf16, tag=f"qs{j}", name=f"qs{j}")
                for j, (b0, n) in enumerate(QS_SLABS)
            ]

            def qs_slab(b):
                for j, (b0, n) in enumerate(QS_SLABS):
                    if b0 <= b < b0 + n:
                        return qsg[j][:, (b - b0) * W:(b - b0 + 1) * W]
                raise AssertionError(b)

            def kick_kv(eng, g):
                c0 = gp0[g] * 2 * NCH * 2 * CA
                eng.dma_start(
                    kvg[g][:], kv_a[:, c0:c0 + PAIR_GROUPS[g] * 2 * NCH * 2 * CA]
                )

            def kick_qs(eng, j):
                b0, n = QS_SLABS[j]
                eng.dma_start(qsg[j][:], qs_a[:, b0 * W:(b0 + n) * W])

            kick_kv(nc.sync, 0)
            nc.scalar.dma_start(cst[:], cst_d.ap()[:])
            kick_kv(nc.scalar, 1)
            kick_kv(nc.sync, 2)
            kick_qs(nc.scalar, 0)
            kick_kv(nc.sync, 4)
            kick_kv(nc.scalar, 3)
            kick_qs(nc.sync, 1)
            kick_qs(nc.scalar, 2)

            # ---- pipelined main loop over block-pairs ----
            gts = [None] * NPAIR     # gt SBUF tiles   [33, 264]
            m12s = [None] * NPAIR    # M1 SBUF tiles   [33, 256]
            wbs = [None] * NPAIR     # W-block SBUF    [128, 264]

            for it in range(NPAIR + PIPE):
                p_m = it - 1   # M1 stage
                p_w = it - 2   # W + bias stage
                p_o = it - 3   # out stage
                p_g = it       # G stage

                # M1: per head, m1 = Gt_h^T @ pvt
                if 0 <= p_m < NPAIR:
                    m1_ps = psm.tile([33, 256], f32, tag="m1")
                    gt = gts[p_m]
                    for hh in range(8):
                        nc.tensor.matmul(
                            m1_ps[:, 32 * hh:32 * (hh + 1)],
                            gt[:, CA * hh:CA * (hh + 1)],
                            pvt,
                        )
                    m12 = spool.tile([33, 256], bf16, tag="m12")
                    m12s[p_m] = m12
                    nc.vector.tensor_copy(m12[:], m1_ps[:])

                # W-blocks: I-init (zeros + residual identity), then
                # per-head diag L = A_w @ M1, plus bias columns
                if 0 <= p_w < NPAIR:
                    w_ps = psw.tile([128, 264], f32, tag="w")
                    m12 = m12s[p_w]
                    nc.tensor.matmul(
                        w_ps[:], i128, ii2,
                        start=True, stop=False, skip_group_check=True,
                    )
                    for blk in range(2):
                        cb = 132 * blk
                        for i in range(4):
                            nc.tensor.matmul(
                                w_ps[32 * i:32 * (i + 1), cb + 32 * i:cb + 32 * (i + 1)],
                                awt,
                                m12[:, 128 * blk + 32 * i:128 * blk + 32 * (i + 1)],
                                start=False, stop=True, skip_group_check=True,
                                tile_position=(0, 32 * i),
                            )
                        nc.tensor.matmul(
                            w_ps[:, cb + 128:cb + 129],
                            m12[:, 128 * blk:128 * (blk + 1)],
                            alast,
                            start=False, stop=True, skip_group_check=True,
                        )
                    wb = spool.tile([128, 264], bf16, tag="wb")
                    wbs[p_w] = wb
                    nc.vector.tensor_copy(wb[:], w_ps[:])

                # out: one K=128 matmul per block + ACTIVATE drain with bias
                if 0 <= p_o < NPAIR:
                    wb = wbs[p_o]
                    osb = opool.tile([128, 2 * W], bf16, tag="osb")
                    for blk in range(2):
                        b = 2 * p_o + blk
                        o_ps = pso.tile([128, W], f32, tag="o")
                        nc.tensor.matmul(
                            o_ps[:],
                            wb[:, 132 * blk:132 * blk + 128],
                            qs_slab(b),
                        )
                        nc.scalar.activation(
                            osb[:, W * blk:W * (blk + 1)],
                            o_ps[:],
                            mybir.ActivationFunctionType.Identity,
                            bias=wb[:, 132 * blk + 128:132 * blk + 129],
                        )
                    if p_o == NPAIR - 1:
                        # split the final flush across both hardware queues
                        nc.sync.dma_start(
                            out_a[:, 2 * p_o * W:(2 * p_o + 1) * W], osb[:, 0:W]
                        )
                        nc.scalar.dma_start(
                            out_a[:, (2 * p_o + 1) * W:2 * (p_o + 1) * W],
                            osb[:, W:2 * W],
                        )
                    else:
                        nc.scalar.dma_start(
                            out_a[:, 2 * p_o * W:2 * (p_o + 1) * W], osb[:]
                        )

                # G: Gt accumulation over 4 chunks per head, 8 heads
                if p_g < NPAIR:
                    g = pair2g[p_g]
                    g_ps = psg.tile([33, 264], f32, tag="g")
                    gt_sb = spool.tile([33, 264], bf16, tag="gt")
                    gts[p_g] = gt_sb
                    ch0 = (p_g - gp0[g]) * 2 * NCH
                    vt0 = PAIR_GROUPS[g] * 2 * NCH * CA  # vt half offset in tile
                    for hh in range(8):
                        for j in range(4):
                            o = (ch0 + hh * 4 + j) * CA
                            nc.tensor.matmul(
                                g_ps[:, CA * hh:CA * (hh + 1)],
                                kvg[g][:, vt0 + o:vt0 + o + CA],
                                kvg[g][:, o:o + CA],
                                start=(j == 0),
                                stop=(j == 3),
                            )
                    nc.vector.tensor_copy(gt_sb[:], g_ps[:])

    nc.compile()
    return nc


def _prep_core(qb, kb, vb):
    """Host relayout for one batch element: qs [128,8192], kv [128,16896]."""
    qs = np.ascontiguousarray(
        qb.reshape(C, NBLK, 4, W).transpose(2, 0, 1, 3)
    ).reshape(128, NBLK * W).astype(_BF16_NP)

    def tr(x):
        t = np.empty((H * W, CA), dtype=np.float32)
        t[:, :C] = x.reshape(C, H * W).T
        t[:, C] = 1.0
        return np.ascontiguousarray(
            t.reshape(NBLK * NCH, 128, CA).transpose(1, 0, 2)
        ).reshape(128, NBLK * NCH, CA)

    # merge per DMA-group: [kt_group | vt_group]
    ktr, vtr = tr(kb), tr(vb)
    kv = np.empty((128, NBLK * NCH * 2 * CA), dtype=np.float32)
    acc = 0
    off = 0
    for n in PAIR_GROUPS:
        cn = n * 2 * NCH * CA
        kv[:, off:off + cn] = ktr[:, acc:acc + cn // CA].reshape(128, cn)
        kv[:, off + cn:off + 2 * cn] = vtr[:, acc:acc + cn // CA].reshape(128, cn)
        acc += cn // CA
        off += 2 * cn
    return qs, kv.astype(_BF16_NP)


def _install_ntff_hook():
    """Provide antenv.axon_hooks (absent in this image) so trace=True works."""
    import sys
    import types

    if "antenv.axon_hooks" in sys.modules:
        return
    try:
        import antenv
    except ImportError:
        return
    mod = types.ModuleType("antenv.axon_hooks")
    store = {}
    mod.set_axon_ntff_profile_hook = lambda h: store.__setitem__("h", h)
    mod.get_axon_ntff_profile_hook = lambda: store.get("h")
    sys.modules["antenv.axon_hooks"] = mod
    antenv.axon_hooks = mod
    try:
        from trn_agent_boot.trn_boot import _ntff_profile_via_ctypes

        hook = _ntff_profile_via_ctypes("/opt/axon/libaxon_pjrt.so")
        if hook is not None:
            store["h"] = hook
    except Exception:
        pass


def kernel(q, k, v, wq, bq, wk, bk, wv, bv):
    global last_exec_time_ns
    if "nc" not in _cache:
        _cache["nc"] = _build()
    nc = _cache["nc"]

    q = np.asarray(q, np.float32)
    k = np.asarray(k, np.float32)
    v = np.asarray(v, np.float32)
    wq = np.asarray(wq, np.float32)
    bq = np.asarray(bq, np.float32)
    wk = np.asarray(wk, np.float32)
    bk = np.asarray(bk, np.float32)
    wv = np.asarray(wv, np.float32)
    bv = np.asarray(bv, np.float32)

    # A = [wq|bq]^T @ [wk|bk]  (33x33), host-side weight algebra
    wqb = np.concatenate([wq, bq[:, None]], axis=1)  # [32, 33]
    wkb = np.concatenate([wk, bk[:, None]], axis=1)
    A = wqb.T @ wkb                                   # [33, 33]
    cst = np.zeros((128, 329), dtype=np.float32)
    cst[0:33, 0:32] = A[0:32, :].T                    # awt
    cst[0:33, 32] = A[32, :]                          # alast
    cst[0:33, 33:65] = np.concatenate([wv.T, bv[None, :]], axis=0)  # pvt
    cst[0:128, 65:193] = np.eye(128)                  # I128
    cst[0:128, 197:325] = np.eye(128)                 # second I128 for ii2
    cst = cst.astype(_BF16_NP)

    in_maps = []
    for b in range(B):
        qs, kv = _prep_core(q[b], k[b], v[b])
        in_maps.append({"qs": qs, "kv": kv, "cst": cst})

    trace = os.environ.get("KERNEL_TRACE", "0") == "1"
    if trace:
        _install_ntff_hook()
    res = run_bass_kernel_spmd(nc, in_maps, core_ids=list(range(B)), trace=trace)
    last_exec_time_ns = res.exec_time_ns

    outs = []
    for b in range(B):
        arr = np.asarray(res.results[b]["out"], dtype=np.float32)
        arr = arr.reshape(4, C, NBLK, W).transpose(1, 2, 0, 3).reshape(C, H, W)
        outs.append(arr)
    return np.stack(outs).astype(np.float32)


# revision 36
# speedup vs baseline: 1.1709x; 1.1709x over previous
"""Trainium2 Bass kernel for nn_MultiHeadAttention_68659347194437.

Spatial attention over the W axis (no softmax) with 1x1-conv projections:
    qp = wq*q + bq ; kp, vp likewise            (C=32 channels)
    attn = qp @ kp^T  per (b,h)                 [512, 512]
    att  = attn @ vp                            [512, 32]
    out  = att^T + q                            (NCHW residual)

No softmax -> the [512,512] score matrix collapses associatively:
    out_h = (A @ M1_h + [I;0])^T @ Qaug_h,   per head h, where
    A  = [wq|bq]^T [wk|bk]                 [33,33]  (host-precomputed)
    M1 = Gt^T Pv^T                         [33,32]  (Gt = Vaug Kaug^T)
The +I folds the residual; row 32 of A@M1 is a per-head output bias.

Device schedule: 8 pipeline iterations of 2 blocks (8 heads). Per block
the final matmul is ONE K=128 matmul with a block-diagonal [128,128]
weight (4 heads stacked) streaming a [128,512] stacked-q tile. The
block-diag weight PSUM is initialized with an identity matmul (zeros
off-diagonal + the residual +I in one PE op), so each PSUM->SBUF drain
is a single wide copy per pair -- DVE ops have ~300ns fixed cost, so
copies are batched aggressively. The per-head bias column rides in the
same copy and feeds the scalar-engine ACTIVATE that drains the output.

Sharding: data-parallel over batch B=8 across 8 NeuronCores, no comms.
Host-side work is pure relayout/packing of inputs and tiny weight algebra.
"""

import os
import numpy as np

import concourse.bass as bass
import concourse.bacc as bacc
import concourse.tile as tile
import concourse.mybir as mybir
from concourse.bass_utils import run_bass_kernel_spmd

B, C, H, W = 8, 32, 64, 512
CA = C + 1           # 33, augmented channel dim
NBLK = H // 4        # 16 blocks of 4 heads
NPAIR = NBLK // 2    # 8 pipeline iterations of 2 blocks
NCH = 4 * 4          # chunks per block (4 heads x 4 chunks of 128 pixels)
PAIR_GROUPS = [1, 1, 2, 2, 2]        # DMA group sizes in block-pairs
PIPE = 3             # out-stage lags G-stage by 3 iterations

last_exec_time_ns = None

_cache = {}

_BF16_NP = np.dtype(mybir.dt.np(mybir.dt.bfloat16))


def _build():
    bf16 = mybir.dt.bfloat16
    f32 = mybir.dt.float32

    nc = bacc.Bacc(
        "TRN2",
        target_bir_lowering=False,
        debug=False,
        enable_asserts=False,
        num_devices=8,
    )

    qs_d = nc.dram_tensor("qs", [128, NBLK * W], bf16, kind="ExternalInput")
    kt_d = nc.dram_tensor("kt", [128, NBLK * NCH * CA], bf16, kind="ExternalInput")
    vt_d = nc.dram_tensor("vt", [128, NBLK * NCH * CA], bf16, kind="ExternalInput")
    # packed consts: cols 0:32 awt (A[0:32,:]^T), 32 alast (A[32,:]),
    # 33:65 pvt ([wv^T; bv]), 65:329 [I128|0|I128|0] (132-col stride)
    cst_d = nc.dram_tensor("cst", [128, 329], bf16, kind="ExternalInput")
    out_d = nc.dram_tensor("out", [128, NBLK * W], bf16, kind="ExternalOutput")

    qs_a = qs_d.ap()
    kt_a = kt_d.ap()
    vt_a = vt_d.ap()
    out_a = out_d.ap()

    gp0 = []
    acc = 0
    for n in PAIR_GROUPS:
        gp0.append(acc)
        acc += n
    pair2g = []
    for g, n in enumerate(PAIR_GROUPS):
        pair2g += [g] * n

    with tile.TileContext(nc) as tc:
        with (
            tc.tile_pool(name="const", bufs=1) as cpool,
            tc.tile_pool(name="inp", bufs=1) as inpool,
            tc.tile_pool(name="small", bufs=3) as spool,
            tc.tile_pool(name="outp", bufs=4) as opool,
            tc.tile_pool(name="psg", bufs=2, space=bass.MemorySpace.PSUM) as psg,
            tc.tile_pool(name="psm", bufs=1, space=bass.MemorySpace.PSUM) as psm,
            tc.tile_pool(name="psw", bufs=2, space=bass.MemorySpace.PSUM) as psw,
            tc.tile_pool(name="pso", bufs=3, space=bass.MemorySpace.PSUM) as pso,
        ):
            cst = cpool.tile([128, 329], bf16)
            awt = cst[0:33, 0:32]       # [33, 32] (A_w^T)
            alast = cst[0:33, 32:33]    # [33, 1]
            pvt = cst[0:33, 33:65]      # [33, 32]
            i128 = cst[:, 65:193]       # [128, 128]
            ii2 = cst[:, 65:329]        # [128, 264]: [I128|0|I128|0]

            # ---- input tiles: kt on sync queue, vt on scalar queue ----
            # pair 0 is split per-block so G(0) starts on the first 135KB
            ktg, vtg = [], []
            for g, n in enumerate(PAIR_GROUPS):
                cn = n * 2 * NCH * CA
                if g == 0:
                    ktg.append([
                        inpool.tile([128, cn // 2], bf16, tag=f"kt0{h}", name=f"kt0{h}")
                        for h in range(2)
                    ])
                    vtg.append([
                        inpool.tile([128, cn // 2], bf16, tag=f"vt0{h}", name=f"vt0{h}")
                        for h in range(2)
                    ])
                else:
                    ktg.append(inpool.tile([128, cn], bf16, tag=f"ktg{g}", name=f"ktg{g}"))
                    vtg.append(inpool.tile([128, cn], bf16, tag=f"vtg{g}", name=f"vtg{g}"))

            def kv_chunk(g, ch, which):
                """AP for chunk `ch` (group-local) of group g."""
                t = ktg[g] if which == 0 else vtg[g]
                if g == 0:
                    half = NCH  # chunks per block
                    t = t[ch // half]
                    ch = ch % half
                return t[:, ch * CA:(ch + 1) * CA]

            # qs in 3 slabs: blocks 0-3, 4-11, 12-15
            QS_SLABS = [(0, 4), (4, 8), (12, 4)]
            qsg = [
                inpool.tile([128, n * W], bf16, tag=f"qs{j}", name=f"qs{j}")
                for j, (b0, n) in enumerate(QS_SLABS)
            ]

            def qs_slab(b):
                for j, (b0, n) in enumerate(QS_SLABS):
                    if b0 <= b < b0 + n:
                        return qsg[j][:, (b - b0) * W:(b - b0 + 1) * W]
                raise AssertionError(b)

            def kick_kv(g):
                c0 = gp0[g] * 2 * NCH * CA
                cn = PAIR_GROUPS[g] * 2 * NCH * CA
                if g == 0:
                    for h in range(2):
                        nc.sync.dma_start(
                            ktg[0][h][:], kt_a[:, h * cn // 2:(h + 1) * cn // 2]
                        )
                        nc.scalar.dma_start(
                            vtg[0][h][:], vt_a[:, h * cn // 2:(h + 1) * cn // 2]
                        )
                else:
                    nc.sync.dma_start(ktg[g][:], kt_a[:, c0:c0 + cn])
                    nc.scalar.dma_start(vtg[g][:], vt_a[:, c0:c0 + cn])

            def kick_qs(eng, j):
                b0, n = QS_SLABS[j]
                eng.dma_start(qsg[j][:], qs_a[:, b0 * W:(b0 + n) * W])

            kick_kv(0)
            nc.scalar.dma_start(cst[:], cst_d.ap()[:])
            kick_kv(1)
            kick_kv(2)
            kick_qs(nc.scalar, 0)
            kick_kv(3)
            kick_qs(nc.sync, 1)
            kick_kv(4)
            kick_qs(nc.scalar, 2)

            # ---- pipelined main loop over block-pairs ----
            gts = [None] * NPAIR     # gt SBUF tiles   [33, 264]
            m12s = [None] * NPAIR    # M1 SBUF tiles   [33, 256]
            wbs = [None] * NPAIR     # W-block SBUF    [128, 264]

            for it in range(NPAIR + PIPE):
                p_m = it - 1   # M1 stage
                p_w = it - 2   # W + bias stage
                p_o = it - 3   # out stage
                p_g = it       # G stage

                # M1: per head, m1 = Gt_h^T @ pvt
                if 0 <= p_m < NPAIR:
                    m1_ps = psm.tile([33, 256], f32, tag="m1")
                    gt = gts[p_m]
                    for hh in range(8):
                        nc.tensor.matmul(
                            m1_ps[:, 32 * hh:32 * (hh + 1)],
                            gt[:, CA * hh:CA * (hh + 1)],
                            pvt,
                        )
                    m12 = spool.tile([33, 256], bf16, tag="m12")
                    m12s[p_m] = m12
                    nc.vector.tensor_copy(m12[:], m1_ps[:])

                # W-blocks: I-init (zeros + residual identity), then
                # per-head diag L = A_w @ M1, plus bias columns
                if 0 <= p_w < NPAIR:
                    w_ps = psw.tile([128, 264], f32, tag="w")
                    m12 = m12s[p_w]
                    nc.tensor.matmul(
                        w_ps[:], i128, ii2,
                        start=True, stop=False, skip_group_check=True,
                    )
                    for blk in range(2):
                        cb = 132 * blk
                        for i in range(4):
                            nc.tensor.matmul(
                                w_ps[32 * i:32 * (i + 1), cb + 32 * i:cb + 32 * (i + 1)],
                                awt,
                                m12[:, 128 * blk + 32 * i:128 * blk + 32 * (i + 1)],
                                start=False, stop=True, skip_group_check=True,
                                tile_position=(0, 32 * i),
                            )
                        nc.tensor.matmul(
                            w_ps[:, cb + 128:cb + 129],
                            m12[:, 128 * blk:128 * (blk + 1)],
                            alast,
                            start=False, stop=True, skip_group_check=True,
                        )
                    wb = spool.tile([128, 264], bf16, tag="wb")
                    wbs[p_w] = wb
                    nc.vector.tensor_copy(wb[:], w_ps[:])

                # out: one K=128 matmul per block + ACTIVATE drain with bias
                if 0 <= p_o < NPAIR:
                    wb = wbs[p_o]
                    osb = opool.tile([128, 2 * W], bf16, tag="osb")
                    for blk in range(2):
                        b = 2 * p_o + blk
                        o_ps = pso.tile([128, W], f32, tag="o")
                        nc.tensor.matmul(
                            o_ps[:],
                            wb[:, 132 * blk:132 * blk + 128],
                            qs_slab(b),
                        )
                        nc.scalar.activation(
                            osb[:, W * blk:W * (blk + 1)],
                            o_ps[:],
                            mybir.ActivationFunctionType.Identity,
                            bias=wb[:, 132 * blk + 128:132 * blk + 129],
                        )
                    if p_o == NPAIR - 1:
                        # split the final flush across both hardware queues
                        nc.sync.dma_start(
                            out_a[:, 2 * p_o * W:(2 * p_o + 1) * W], osb[:, 0:W]
                        )
                        nc.scalar.dma_start(
                            out_a[:, (2 * p_o + 1) * W:2 * (p_o + 1) * W],
                            osb[:, W:2 * W],
                        )
                    else:
                        nc.scalar.dma_start(
                            out_a[:, 2 * p_o * W:2 * (p_o + 1) * W], osb[:]
                        )

                # G: Gt accumulation over 4 chunks per head, 8 heads
                if p_g < NPAIR:
                    g = pair2g[p_g]
                    g_ps = psg.tile([33, 264], f32, tag="g")
                    gt_sb = spool.tile([33, 264], bf16, tag="gt")
                    gts[p_g] = gt_sb
                    ch0 = (p_g - gp0[g]) * 2 * NCH
                    for hh in range(8):
                        for j in range(4):
                            ch = ch0 + hh * 4 + j
                            nc.tensor.matmul(
                                g_ps[:, CA * hh:CA * (hh + 1)],
                                kv_chunk(g, ch, 1),
                                kv_chunk(g, ch, 0),
                                start=(j == 0),
                                stop=(j == 3),
                            )
                    nc.vector.tensor_copy(gt_sb[:], g_ps[:])

    nc.compile()
    return nc


def _prep_core(qb, kb, vb):
    """Host relayout for one batch element: qs [128,8192], kt/vt [128,8448]."""
    qs = np.ascontiguousarray(
        qb.reshape(C, NBLK, 4, W).transpose(2, 0, 1, 3)
    ).reshape(128, NBLK * W).astype(_BF16_NP)

    def tr(x):
        t = np.empty((H * W, CA), dtype=np.float32)
        t[:, :C] = x.reshape(C, H * W).T
        t[:, C] = 1.0
        return np.ascontiguousarray(
            t.reshape(NBLK * NCH, 128, CA).transpose(1, 0, 2)
        ).reshape(128, NBLK * NCH * CA).astype(_BF16_NP)

    return qs, tr(kb), tr(vb)


def _install_ntff_hook():
    """Provide antenv.axon_hooks (absent in this image) so trace=True works."""
    import sys
    import types

    if "antenv.axon_hooks" in sys.modules:
        return
    try:
        import antenv
    except ImportError:
        return
    mod = types.ModuleType("antenv.axon_hooks")
    store = {}
    mod.set_axon_ntff_profile_hook = lambda h: store.__setitem__("h", h)
    mod.get_axon_ntff_profile_hook = lambda: store.get("h")
    sys.modules["antenv.axon_hooks"] = mod
    antenv.axon_hooks = mod
    try:
        from trn_agent_boot.trn_boot import _ntff_profile_via_ctypes

        hook = _ntff_profile_via_ctypes("/opt/axon/libaxon_pjrt.so")
        if hook is not None:
            store["h"] = hook
    except Exception:
        pass


def kernel(q, k, v, wq, bq, wk, bk, wv, bv):
    global last_exec_time_ns
    if "nc" not in _cache:
        _cache["nc"] = _build()
    nc = _cache["nc"]

    q = np.asarray(q, np.float32)
    k = np.asarray(k, np.float32)
    v = np.asarray(v, np.float32)
    wq = np.asarray(wq, np.float32)
    bq = np.asarray(bq, np.float32)
    wk = np.asarray(wk, np.float32)
    bk = np.asarray(bk, np.float32)
    wv = np.asarray(wv, np.float32)
    bv = np.asarray(bv, np.float32)

    # A = [wq|bq]^T @ [wk|bk]  (33x33), host-side weight algebra
    wqb = np.concatenate([wq, bq[:, None]], axis=1)  # [32, 33]
    wkb = np.concatenate([wk, bk[:, None]], axis=1)
    A = wqb.T @ wkb                                   # [33, 33]
    cst = np.zeros((128, 329), dtype=np.float32)
    cst[0:33, 0:32] = A[0:32, :].T                    # awt
    cst[0:33, 32] = A[32, :]                          # alast
    cst[0:33, 33:65] = np.concatenate([wv.T, bv[None, :]], axis=0)  # pvt
    cst[0:128, 65:193] = np.eye(128)                  # I128
    cst[0:128, 197:325] = np.eye(128)                 # second I128 for ii2
    cst = cst.astype(_BF16_NP)

    in_maps = []
    for b in range(B):
        qs, kt, vt = _prep_core(q[b], k[b], v[b])
        in_maps.append({"qs": qs, "kt": kt, "vt": vt, "cst": cst})

    trace = os.environ.get("KERNEL_TRACE", "0") == "1"
    if trace:
        _install_ntff_hook()
    res = run_bass_kernel_spmd(nc, in_maps, core_ids=list(range(B)), trace=trace)
    last_exec_time_ns = res.exec_time_ns

    outs = []
    for b in range(B):
        arr = np.asarray(res.results[b]["out"], dtype=np.float32)
        arr = arr.reshape(4, C, NBLK, W).transpose(1, 2, 0, 3).reshape(C, H, W)
        outs.append(arr)
    return np.stack(outs).astype(np.float32)
